# revision 26
# baseline (speedup 1.0000x reference)
"""Sequence-parallel attention kernel for 8 Trainium2 NeuronCores — v4.

Problem: nn_Attention_v2 — QKV projections + softmax attention + out-proj.
  query [2048,256], key/value [16384,256], weights [256,256], H=8 heads, KD=VD=32.

Sharding: K/V sequence split 8 ways (2048 rows/core); query replicated.
Each core computes, for all 8 heads, the *unnormalized* attention numerator
Onum = exp(S) @ V and denominator l = exp(S) @ 1 over its local K/V chunk
(logits are bounded ~|S|<10, exp is safe without max subtraction).
A ReduceScatter sums (Onum, l) across cores and shards the result by query
columns; each core then divides, applies the output projection for its query
shard, and the host concatenates the 8 shards.

v4 structure (vs v2):
- The whole matmul path is fp16 (weights, kT/qT/vT, khT/qhT, vh, P, attnT):
  fp16 streams 1 PE cycle/row where f32r streams 2, halving the S quad from
  ~930ns to ~470ns and the projections likewise. fp16's 10 mantissa bits keep
  logit error ~1e-3, far below the fast-exp ripple.
- exp optionally split across THREE engines (POOL_EXP): DVE fast-exps X cols
  0:XD, GPSIMD(Pool) fast-exps X[XD:] + Y[:YP], ACT exact-exps Y[YP:]. Pool
  runs at 1.2GHz like ACT; 3-way brings the per-step exp wall from ~1.2us
  down to ~0.85us. Falls back to the v2 DVE/ACT halves when POOL_EXP=False.
- Per-PAIR epilogue: segments (2jq, 2jq+1) cover both quads for q block jq,
  so divide + out-proj + store run per 64-row block as soon as that pair's
  ReduceScatter lands. tile_wait_until parks every RS-dependent epilogue op
  late in the scheduler's model time so none of them is queue-ordered before
  remaining main-loop ops (v2 lost ~19us to the seg-6 recip head-of-line
  blocking DVE's queue mid-segment-7, plus ~26us of serialized tail).
- Denominator reciprocal is broadcast to the 32 rows of each head by a tiny
  f32r matmul with a 0/1 block mask (PE, ~107ns) instead of 8 GpSimd
  partition_broadcasts, keeping Pool free for exp work.
- S tiles rotate through 3 PSUM half-tiles [128,1024]; PSUM: 3*2(S) + 1(psO)
  + 1(den) = 8 banks exactly. Prologue transposes/projections are interleaved
  into the first main-loop steps as in v2.
"""
import sys

sys.path.insert(0, "/opt/trn_rl_repo")

import numpy as np

import concourse.bass as bass  # noqa: F401  (import order matters)
from concourse import bacc
import concourse.mybir as mybir
from concourse.bass_utils import run_bass_kernel_spmd
from concourse.tile import TileContext
from concourse.masks import make_identity

F32 = mybir.dt.float32
F32R = mybir.dt.float32r
I32 = mybir.dt.int32
F16 = mybir.dt.float16
I16 = mybir.dt.int16
EXP = mybir.ActivationFunctionType.Exp
COPY = mybir.ActivationFunctionType.Copy

NC_CORES = 8
TQ, T, D = 2048, 16384, 256
H, KD, VD, DOUT = 8, 32, 32, 256
HD = H * KD  # 256
TLOC = T // NC_CORES          # 2048 local K/V rows
NKT = TLOC // 128             # 16 k-chunks
NJQ = TQ // 512               # 4 q-column chunks of 512
QG = 64                       # q columns per rank-group in the RS layout
SCALE = float(1.0 / np.sqrt(KD))

# exp engine split (columns of the 2048-wide P per step):
#   DVE fast-exp: X[:, 0:XD]          (1.042 ns/col)
#   Pool fast-exp: X[:, XD:1024] + Y[:, 0:YP]   (0.833 ns/col)
#   ACT exact exp: Y[:, YP:1024]      (0.833 ns/col)
POOL_EXP = False  # GPSIMD cannot access PSUM (BIR verifier) — 2-way split only
XD = 640
YP = 384

# Schraudolph fast-exp in fp16: bits(int16(A*s + B)) viewed as fp16 ~= exp(s)
# (multiplicative ripple up to +-3.6%; the softmax num/denom ratio cancels
# most of it -- measured 1.09e-2 end-to-end max rel vs the 2e-2 gate in v2).
A_EXP = float(2 ** 10 / np.log(2.0))
B_EXP = float(15.0 * 2 ** 10 - 292498.0 / 8192.0)  # centered ripple


def build_nc():
    nc = bacc.Bacc("TRN2", target_bir_lowering=False)

    t_query = nc.dram_tensor("query", [TQ, D], F32, kind="ExternalInput")
    t_key = nc.dram_tensor("key", [TLOC, D], F32, kind="ExternalInput")
    t_value = nc.dram_tensor("value", [TLOC, D], F32, kind="ExternalInput")
    t_wq = nc.dram_tensor("wq", [D, HD], F32, kind="ExternalInput")
    t_wk = nc.dram_tensor("wk", [D, HD], F32, kind="ExternalInput")
    t_wv = nc.dram_tensor("wv", [D, HD], F32, kind="ExternalInput")
    t_wo = nc.dram_tensor("wo", [HD, DOUT], F32, kind="ExternalInput")
    t_bq = nc.dram_tensor("bq", [HD], F32, kind="ExternalInput")
    t_bk = nc.dram_tensor("bk", [HD], F32, kind="ExternalInput")
    t_bv = nc.dram_tensor("bv", [HD], F32, kind="ExternalInput")
    t_bo = nc.dram_tensor("bo", [DOUT], F32, kind="ExternalInput")
    t_out = nc.dram_tensor("out", [TQ // NC_CORES, DOUT], F32, kind="ExternalOutput")

    with TileContext(nc) as tc:
        with tc.tile_pool(name="const", bufs=1) as constp, \
             tc.tile_pool(name="persist", bufs=1) as persist, \
             tc.tile_pool(name="ep", bufs=1) as ep, \
             tc.tile_pool(name="dram", bufs=1, space="DRAM") as dramp:

            ident = constp.tile([128, 128], F32)
            make_identity(nc, ident[:])

            # warm the ACT exp table before the main loop needs it
            dummy = constp.tile([128, 32], F32)
            nc.gpsimd.memset(dummy[:], 0.0)
            dexp = constp.tile([128, 32], F32R)
            nc.scalar.activation(dexp[:], dummy[:], EXP)

            # 0/1 block mask for the denominator broadcast matmul:
            # bcast4[j, 32j:32j+32] = 1 -> out[m, q] = rec[m//32, q].
            # Built as its transpose (memsets at 32-aligned partition bases,
            # which the ISA requires) and flipped once on the PE.
            bcT = constp.tile([128, 4], F32)
            nc.gpsimd.memset(bcT[:], 0.0)
            for j in range(4):
                nc.gpsimd.memset(bcT[32 * j:32 * (j + 1), j:j + 1], 1.0)
            bcast4 = constp.tile([4, 128], F32R)
            with tc.tile_pool(name="pBC", bufs=1, space="PSUM") as pBC:
                bc_ps = pBC.tile([4, 128], F32, tag="bcps", name="bcps")
                nc.tensor.transpose(bc_ps[:], bcT[:], ident[:])
                nc.vector.tensor_copy(bcast4[:], bc_ps[:])

            # persistent projected tensors (quad layout: tile m = heads
            # 4m..4m+3, head h at rows 32*(h%4)..)
            qhT = [persist.tile([128, TQ], F16, tag=f"qhT{m}", name=f"qhT{m}") for m in range(2)]
            khT = [persist.tile([128, TLOC], F16, tag=f"khT{m}", name=f"khT{m}") for m in range(2)]
            vh = [persist.tile([128, 256], F16, tag=f"vh{t}", name=f"vh{t}") for t in range(NKT)]
            # one shared all-ones column feeds every denominator matmul
            ones_col = persist.tile([128, 1], F16, tag="onescol", name="onescol")
            nc.gpsimd.memset(ones_col[:], 1.0)
            # epilogue accumulators (filled by per-segment RS-output DMAs)
            osum = [ep.tile([128, 256], F32, tag=f"osum{m}", name=f"osum{m}") for m in range(2)]
            # denominators packed [4 heads, t*256 + jq*64 + c]
            ldnP = ep.tile([4, 512], F32, tag="ldnP", name="ldnP")
            nc.gpsimd.memset(ldnP[:], 1.0)

            with tc.tile_pool(name="tin", bufs=6) as tin, \
                 tc.tile_pool(name="t16", bufs=6) as t16p, \
                 tc.tile_pool(name="tT", bufs=1) as tTp, \
                 tc.tile_pool(name="pS", bufs=3, space="PSUM") as pS, \
                 tc.tile_pool(name="pO", bufs=1, space="PSUM") as pO, \
                 tc.tile_pool(name="pD", bufs=1, space="PSUM") as pD, \
                 tc.tile_pool(name="pP", bufs=3) as pP, \
                 tc.tile_pool(name="stage", bufs=4) as stage:

                pre_raw = {}

                def load_raw(tdram, i, tag):
                    raw = tin.tile([128, 256], F32, tag=f"in_{tag}", name=f"in_{tag}")
                    nc.sync.dma_start(out=raw[:], in_=tdram[i * 128:(i + 1) * 128, :])
                    return raw

                for _i in range(4):
                    pre_raw[("kT", _i)] = load_raw(t_key, _i, "kT")

                # ---- weights + biases to SBUF (fp16; wq,bq pre-scaled by
                # 1/sqrt(KD)). One DMA per weight: [256,256] -> [128, 512]
                # with D-chunk a in cols 256a.. ----
                wcomb = {}
                with tc.tile_pool(name="wstage", bufs=2) as wstage:
                    for (tdram, key, scale_mul) in ((t_wk, "wk", None), (t_wq, "wq", SCALE),
                                                    (t_wv, "wv", None), (t_wo, "wo", None)):
                        raw = wstage.tile([128, 512], F32, tag="wraw", name="wraw")
                        nc.sync.dma_start(
                            out=raw[:].rearrange("p (a d) -> p a d", a=2),
                            in_=tdram[:].rearrange("(a p) d -> p a d", a=2))
                        wt = persist.tile([128, 512], F16, tag=f"w_{key}", name=f"w_{key}")
                        if scale_mul is not None:
                            nc.vector.tensor_scalar_mul(wt[:], raw[:], scale_mul)
                        else:
                            nc.vector.tensor_copy(wt[:], raw[:])
                        wcomb[key] = wt
                    wk_r = [wcomb["wk"][:, 256 * dc:256 * (dc + 1)] for dc in range(2)]
                    wq_r = [wcomb["wq"][:, 256 * dc:256 * (dc + 1)] for dc in range(2)]
                    wv_r = [wcomb["wv"][:, 256 * dc:256 * (dc + 1)] for dc in range(2)]
                    wo_r = [wcomb["wo"][:, 256 * dc:256 * (dc + 1)] for dc in range(2)]
                    for _i in range(4):
                        pre_raw[("qT", _i)] = load_raw(t_query, _i, "qT")
                    bq_c, bk_c, bo_c = [None, None], [None, None], [None, None]
                    for (tdram, dst, scale_mul, key) in ((t_bk, bk_c, None, "bk"),
                                                         (t_bq, bq_c, SCALE, "bq"),
                                                         (t_bo, bo_c, None, "bo")):
                        braw = wstage.tile([128, 2], F32, tag="braw", name="braw")
                        nc.sync.dma_start(out=braw[:],
                                          in_=tdram[:].rearrange("(a p) -> p a", a=2))
                        bt = persist.tile([128, 2], F32, tag=f"b_{key}", name=f"b_{key}")
                        if scale_mul is not None:
                            nc.vector.tensor_scalar_mul(bt[:], braw[:], scale_mul)
                        else:
                            nc.vector.tensor_copy(bt[:], braw[:])
                        for m in range(2):
                            dst[m] = bt[:, m:m + 1]
                    # bv replicated across partitions for the vh epilogue
                    bv_row = persist.tile([1, 256], F32)
                    nc.sync.dma_start(out=bv_row[:], in_=t_bv[:].rearrange("(a d) -> a d", a=1))
                    bv_rep = persist.tile([128, 256], F32)
                    nc.gpsimd.partition_broadcast(bv_rep[:], bv_row[0:1, :])

                qT = [tTp.tile([128, TQ], F16, tag=f"qT{m}", name=f"qT{m}") for m in range(2)]
                kT = [tTp.tile([128, TLOC], F16, tag=f"kT{m}", name=f"kT{m}") for m in range(2)]
                vT = [tTp.tile([128, TLOC], F16, tag=f"vT{m}", name=f"vT{m}") for m in range(2)]

                def load4_transpose(tdram, dst, j, tag):
                    """Load 4 raw [128,256] f32 tiles (rows 512j..), cast to
                    fp16 (casts alternate DVE/ACT), then transpose into
                    dst[dc][:, 512j:512j+512] with XBAR transposing DMAs —
                    no PE time at all."""
                    for ti in range(4):
                        i = 4 * j + ti
                        raw = pre_raw.pop((tag, i), None)
                        if raw is None:
                            raw = load_raw(tdram, i, tag)
                        r16 = t16p.tile([128, 256], F16, tag=f"r16_{tag}",
                                        name=f"r16_{tag}")
                        if ti % 2 == 0:
                            nc.vector.tensor_copy(r16[:], raw[:])
                        else:
                            nc.scalar.activation(r16[:], raw[:], COPY)
                        for dc in range(2):
                            nc.sync.dma_start_transpose(
                                out=dst[dc][:, 512 * j + 128 * ti:512 * j + 128 * (ti + 1)],
                                in_=r16[:, 128 * dc:128 * (dc + 1)])

                def kchunk(j):
                    load4_transpose(t_key, kT, j, "kT")
                    pp = pS.tile([128, 1024], F32, tag="S", name="projk")
                    for m in range(2):
                        for dc in range(2):
                            nc.tensor.matmul(pp[:, m * 512:(m + 1) * 512],
                                             wk_r[dc][:, m * 128:(m + 1) * 128],
                                             kT[dc][:, j * 512:(j + 1) * 512],
                                             start=(dc == 0), stop=(dc == 1))
                    for m in range(2):
                        nc.vector.tensor_scalar_add(khT[m][:, j * 512:(j + 1) * 512],
                                                    pp[:, m * 512:(m + 1) * 512], bk_c[m])

                def qchunk(j):
                    load4_transpose(t_query, qT, j, "qT")
                    pp = pS.tile([128, 1024], F32, tag="S", name="projq")
                    for m in range(2):
                        for dc in range(2):
                            nc.tensor.matmul(pp[:, m * 512:(m + 1) * 512],
                                             wq_r[dc][:, m * 128:(m + 1) * 128],
                                             qT[dc][:, j * 512:(j + 1) * 512],
                                             start=(dc == 0), stop=(dc == 1))
                    for m in range(2):
                        nc.vector.tensor_scalar_add(qhT[m][:, j * 512:(j + 1) * 512],
                                                    pp[:, m * 512:(m + 1) * 512], bq_c[m])

                def vchunk(j):
                    load4_transpose(t_value, vT, j, "vT")
                    pp = pS.tile([128, 1024], F32, tag="S", name="projv")
                    for ti in range(4):
                        t = 4 * j + ti
                        for dc in range(2):
                            nc.tensor.matmul(pp[:, ti * 256:(ti + 1) * 256],
                                             vT[dc][:, t * 128:(t + 1) * 128],
                                             wv_r[dc][:], start=(dc == 0), stop=(dc == 1))
                    for ti in range(4):
                        t = 4 * j + ti
                        nc.vector.tensor_add(vh[t][:], pp[:, ti * 256:(ti + 1) * 256],
                                             bv_rep[:])

                # ---- main loop: 8 segments (jq, quad) x 16 kk, flat steps ----
                NSTEP = NJQ * 2 * NKT  # 128
                z_in = [dramp.tile([NC_CORES, 132, QG], F32, tag=f"zi{si}",
                                   name=f"zi{si}") for si in range(NJQ * 2)]
                z_out = [dramp.tile([132, QG], F32, tag=f"zo{si}",
                                    name=f"zo{si}") for si in range(NJQ * 2)]

                def step_seg(i):
                    return i // NKT  # segment index

                def seg_jq_t(si):
                    return si // 2, si % 2

                S_tiles = {}   # step -> (X, Y)
                P_tiles = {}   # step -> P
                psO_cur = [None]
                psD_cur = [None]

                def emit_S(i):
                    jq, t = seg_jq_t(step_seg(i))
                    kk = i % NKT
                    X = pS.tile([128, 1024], F32, tag="S", name="Sx")
                    Y = pS.tile([128, 1024], F32, tag="S", name="Sy")
                    for j in range(2):
                        nc.tensor.matmul(X[:, j * 512:(j + 1) * 512],
                                         khT[t][32 * j:32 * j + 32, kk * 128:(kk + 1) * 128],
                                         qhT[t][32 * j:32 * j + 32, jq * 512:(jq + 1) * 512],
                                         start=True, stop=True, tile_position=(32 * j, 0))
                    for j in range(2, 4):
                        nc.tensor.matmul(Y[:, (j - 2) * 512:(j - 1) * 512],
                                         khT[t][32 * j:32 * j + 32, kk * 128:(kk + 1) * 128],
                                         qhT[t][32 * j:32 * j + 32, jq * 512:(jq + 1) * 512],
                                         start=True, stop=True, tile_position=(32 * j, 0))
                    S_tiles[i] = (X, Y)

                def emit_exp(i):
                    X, Y = S_tiles.pop(i)
                    P = pP.tile([128, 2048], F16, tag="P", name="P")
                    if POOL_EXP:
                        nc.vector.tensor_scalar(
                            out=P[:, 0:XD].bitcast(I16), in0=X[:, 0:XD],
                            scalar1=A_EXP, scalar2=B_EXP,
                            op0=mybir.AluOpType.mult, op1=mybir.AluOpType.add)
                        nc.gpsimd.tensor_scalar(
                            out=P[:, XD:1024].bitcast(I16), in0=X[:, XD:1024],
                            scalar1=A_EXP, scalar2=B_EXP,
                            op0=mybir.AluOpType.mult, op1=mybir.AluOpType.add)
                        nc.gpsimd.tensor_scalar(
                            out=P[:, 1024:1024 + YP].bitcast(I16), in0=Y[:, 0:YP],
                            scalar1=A_EXP, scalar2=B_EXP,
                            op0=mybir.AluOpType.mult, op1=mybir.AluOpType.add)
                        nc.scalar.activation(P[:, 1024 + YP:2048], Y[:, YP:1024], EXP)
                    else:
                        nc.vector.tensor_scalar(
                            out=P[:, 0:1024].bitcast(I16), in0=X[:],
                            scalar1=A_EXP, scalar2=B_EXP,
                            op0=mybir.AluOpType.mult, op1=mybir.AluOpType.add)
                        nc.scalar.activation(P[:, 1024:2048], Y[:], EXP)
                    P_tiles[i] = P

                def emit_AV(i):
                    si = step_seg(i)
                    jq, t = seg_jq_t(si)
                    kk = i % NKT
                    first, last = kk == 0, kk == NKT - 1
                    if first:
                        psO_cur[0] = pO.tile([128, 512], F32, tag="psO", name="psO")
                        psD_cur[0] = pD.tile([128, 512], F32, tag="psD", name="psD")
                    P = P_tiles.pop(i)
                    psO, psD = psO_cur[0], psD_cur[0]
                    for j in range(4):
                        h = 4 * t + j
                        nc.tensor.matmul(psO[32 * j:32 * j + 32, :],
                                         vh[kk][:, 32 * h:32 * h + 32],
                                         P[:, j * 512:(j + 1) * 512],
                                         start=first, stop=last,
                                         tile_position=(0, 32 * j), skip_group_check=True)
                    for j in range(4):
                        nc.tensor.matmul(psD[32 * j:32 * j + 1, :],
                                         ones_col[:],
                                         P[:, j * 512:(j + 1) * 512],
                                         start=first, stop=last,
                                         tile_position=(0, 32 * j), skip_group_check=True)
                    return (psO, psD) if last else None

                def emit_drain(si, psO, psD):
                    stO = stage.tile([128, 512], F32, tag="stO", name="stO")
                    stD = stage.tile([128, 512], F32, tag="stD", name="stD")
                    # both drain copies on ACT: DVE is the steady-state
                    # bottleneck (fast-exp 1.23us/step), ACT has ~90ns slack
                    nc.scalar.activation(stO[:], psO[:], COPY)
                    nc.scalar.activation(stD[:], psD[:], COPY)
                    zi = z_in[si]
                    nc.sync.dma_start(
                        out=zi[:, 0:128, :].rearrange("r p c -> p r c"),
                        in_=stO[:].rearrange("p (r c) -> p r c", r=NC_CORES))
                    nc.sync.dma_start(
                        out=zi[:, 128:132, :].rearrange("r p c -> p r c"),
                        in_=stD[0:128:32, :].rearrange("p (r c) -> p r c", r=NC_CORES))
                    nc.gpsimd.collective_compute(
                        "ReduceScatter", mybir.AluOpType.add,
                        replica_groups=[list(range(NC_CORES))],
                        ins=[zi.opt()], outs=[z_out[si].opt()])

                # prologue chunks interleaved into the first steps: chunk
                # emitted at step i is consumed from step ~i+2 onward.
                prologue_at = {
                    0: lambda: kchunk(1), 1: lambda: vchunk(1),
                    2: lambda: kchunk(2), 3: lambda: vchunk(2),
                    5: lambda: kchunk(3), 7: lambda: vchunk(3),
                    10: lambda: qchunk(1), 16: lambda: qchunk(2),
                    24: lambda: qchunk(3),
                }

                kchunk(0)
                qchunk(0)
                emit_S(0)
                vchunk(0)
                pending = None
                for i in range(NSTEP):
                    if i in prologue_at:
                        prologue_at[i]()
                    emit_exp(i)
                    if i + 1 < NSTEP:
                        emit_S(i + 1)
                    if pending is not None:
                        emit_drain(*pending)
                        pending = None
                    fin = emit_AV(i)
                    if fin is not None:
                        pending = (step_seg(i), fin[0], fin[1])
                emit_drain(*pending)

            # ---- per-pair epilogue: as soon as segments (2jq, 2jq+1) have
            # been reduce-scattered, normalize + out-project + store the
            # 64 q rows this core owns in block jq. All of it is parked late
            # in the scheduler's model time so no queue orders it before
            # remaining main-loop work (head-of-line blocking). ----
            with tc.tile_pool(name="pRL", bufs=1, space="PSUM") as pRL, \
                 tc.tile_pool(name="pPO", bufs=1, space="PSUM") as pPO, \
                 tc.tile_pool(name="pPT", bufs=1, space="PSUM") as pPT, \
                 tc.tile_pool(name="epr", bufs=2) as epr:
                attnT = [ep.tile([128, 256], F16, tag=f"attnT{m}", name=f"attnT{m}")
                         for m in range(2)]
                for jq in range(NJQ):
                    with tc.tile_wait_until(0.24 + 0.001 * jq):
                        cols = slice(jq * QG, (jq + 1) * QG)
                        for t in range(2):
                            si = 2 * jq + t
                            zo = z_out[si]
                            nc.sync.dma_start(
                                out=osum[t][:, cols], in_=zo[0:128, :])
                            nc.sync.dma_start(
                                out=ldnP[0:4, 256 * t + jq * QG:256 * t + (jq + 1) * QG],
                                in_=zo[128:132, :])
                        # reciprocal of both quads' dens in one strided op,
                        # then one K=4 mask-matmul per quad broadcasts each
                        # head's recip row to its 32 (head,vd) output rows
                        rec = epr.tile([4, 128], F32R, tag="rec", name="rec")
                        with nc.allow_low_precision(reason="f32r recip feeds matmul"):
                            nc.vector.reciprocal(
                                rec[:].rearrange("p (t c) -> p t c", t=2),
                                ldnP[:].rearrange("p (t j c) -> p t j c", t=2, j=NJQ)[:, :, jq, :])
                        rlP = [pRL.tile([128, 64], F32, tag=f"rlP{t}", name=f"rlP{t}")
                               for t in range(2)]
                        for t in range(2):
                            nc.tensor.matmul(rlP[t][:], bcast4[:],
                                             rec[:, 64 * t:64 * (t + 1)],
                                             start=True, stop=True)
                        for t in range(2):
                            nc.vector.tensor_mul(attnT[t][:, cols],
                                                 osum[t][:, cols], rlP[t][:])
                        # out-proj for this q block: one PSUM tile per dout
                        # chunk; m=0,1 accumulate into it.
                        pout = [pPO.tile([128, 64], F32, tag=f"pout{dc}", name=f"pout{dc}")
                                for dc in range(2)]
                        for dc in range(2):
                            for m in range(2):
                                nc.tensor.matmul(pout[dc][:],
                                                 wo_r[m][:, dc * 128:(dc + 1) * 128],
                                                 attnT[m][:, cols], start=(m == 0),
                                                 stop=(m == 1), skip_group_check=True)
                        oT = epr.tile([128, 128], F32, tag="oT", name="oT")
                        for dc in range(2):
                            nc.vector.tensor_scalar_add(oT[:, 64 * dc:64 * (dc + 1)],
                                                        pout[dc][:], bo_c[dc])
                        pt = [pPT.tile([64, 128], F32, tag=f"pt{dc}", name=f"pt{dc}")
                              for dc in range(2)]
                        for dc in range(2):
                            nc.tensor.transpose(pt[dc][:],
                                                oT[:, 64 * dc:64 * (dc + 1)], ident[:])
                        out_sb = epr.tile([64, 256], F32, tag="outsb", name="outsb")
                        for dc in range(2):
                            nc.vector.tensor_copy(out_sb[:, 128 * dc:128 * (dc + 1)],
                                                  pt[dc][:])
                        nc.sync.dma_start(out=t_out[jq * QG:(jq + 1) * QG, :],
                                          in_=out_sb[:])

    nc.compile()
    return nc


_NC_CACHE = {}


def _get_nc():
    if "nc" not in _NC_CACHE:
        _NC_CACHE["nc"] = build_nc()
    return _NC_CACHE["nc"]


def run_cores(inputs, trace=False):
    nc = _get_nc()
    full = {k: np.ascontiguousarray(np.asarray(v, dtype=np.float32)) for k, v in inputs.items()}
    in_maps = []
    for c in range(NC_CORES):
        m = dict(full)
        m["key"] = np.ascontiguousarray(full["key"][c * TLOC:(c + 1) * TLOC])
        m["value"] = np.ascontiguousarray(full["value"][c * TLOC:(c + 1) * TLOC])
        in_maps.append(m)
    res = run_bass_kernel_spmd(nc, in_maps, core_ids=list(range(NC_CORES)), trace=trace)
    out = np.empty((TQ, DOUT), dtype=np.float32)
    for r in range(NC_CORES):
        blk = res.results[r]["out"]
        for jq in range(NJQ):
            q0 = QG * (NC_CORES * jq + r)
            out[q0:q0 + QG, :] = blk[QG * jq:QG * (jq + 1), :]
    return out, res


def kernel(**inputs) -> np.ndarray:
    out, _ = run_cores(inputs, trace=False)
    return out


# revision 27
# speedup vs baseline: 1.8521x; 1.8521x over previous
"""Sequence-parallel attention kernel for 8 Trainium2 NeuronCores — v4.

Problem: nn_Attention_v2 — QKV projections + softmax attention + out-proj.
  query [2048,256], key/value [16384,256], weights [256,256], H=8 heads, KD=VD=32.

Sharding: K/V sequence split 8 ways (2048 rows/core); query replicated.
Each core computes, for all 8 heads, the *unnormalized* attention numerator
Onum = exp(S) @ V and denominator l = exp(S) @ 1 over its local K/V chunk
(logits are bounded ~|S|<10, exp is safe without max subtraction).
A ReduceScatter sums (Onum, l) across cores and shards the result by query
columns; each core then divides, applies the output projection for its query
shard, and the host concatenates the 8 shards.

v4 structure (vs v2):
- The whole matmul path is fp16 (weights, kT/qT/vT, khT/qhT, vh, P, attnT):
  fp16 streams 1 PE cycle/row where f32r streams 2, halving the S quad from
  ~930ns to ~470ns and the projections likewise. fp16's 10 mantissa bits keep
  logit error ~1e-3, far below the fast-exp ripple.
- exp optionally split across THREE engines (POOL_EXP): DVE fast-exps X cols
  0:XD, GPSIMD(Pool) fast-exps X[XD:] + Y[:YP], ACT exact-exps Y[YP:]. Pool
  runs at 1.2GHz like ACT; 3-way brings the per-step exp wall from ~1.2us
  down to ~0.85us. Falls back to the v2 DVE/ACT halves when POOL_EXP=False.
- Per-PAIR epilogue: segments (2jq, 2jq+1) cover both quads for q block jq,
  so divide + out-proj + store run per 64-row block as soon as that pair's
  ReduceScatter lands. tile_wait_until parks every RS-dependent epilogue op
  late in the scheduler's model time so none of them is queue-ordered before
  remaining main-loop ops (v2 lost ~19us to the seg-6 recip head-of-line
  blocking DVE's queue mid-segment-7, plus ~26us of serialized tail).
- Denominator reciprocal is broadcast to the 32 rows of each head by a tiny
  f32r matmul with a 0/1 block mask (PE, ~107ns) instead of 8 GpSimd
  partition_broadcasts, keeping Pool free for exp work.
- S tiles rotate through 3 PSUM half-tiles [128,1024]; PSUM: 3*2(S) + 1(psO)
  + 1(den) = 8 banks exactly. Prologue transposes/projections are interleaved
  into the first main-loop steps as in v2.
"""
import sys

sys.path.insert(0, "/opt/trn_rl_repo")

import numpy as np

import concourse.bass as bass  # noqa: F401  (import order matters)
from concourse import bacc
import concourse.mybir as mybir
from concourse.bass_utils import run_bass_kernel_spmd
from concourse.tile import TileContext
from concourse.masks import make_identity

F32 = mybir.dt.float32
F32R = mybir.dt.float32r
I32 = mybir.dt.int32
F16 = mybir.dt.float16
I16 = mybir.dt.int16
EXP = mybir.ActivationFunctionType.Exp
COPY = mybir.ActivationFunctionType.Copy

NC_CORES = 8
TQ, T, D = 2048, 16384, 256
H, KD, VD, DOUT = 8, 32, 32, 256
HD = H * KD  # 256
TLOC = T // NC_CORES          # 2048 local K/V rows
NKT = TLOC // 128             # 16 k-chunks
NJQ = TQ // 512               # 4 q-column chunks of 512
QG = 64                       # q columns per rank-group in the RS layout
SCALE = float(1.0 / np.sqrt(KD))

# exp engine split (columns of the 2048-wide P per step):
#   DVE fast-exp: X[:, 0:XD]          (1.042 ns/col)
#   Pool fast-exp: X[:, XD:1024] + Y[:, 0:YP]   (0.833 ns/col)
#   ACT exact exp: Y[:, YP:1024]      (0.833 ns/col)
POOL_EXP = False  # GPSIMD cannot access PSUM (BIR verifier) — 2-way split only
XD = 640
YP = 384

# Schraudolph fast-exp in fp16: bits(int16(A*s + B)) viewed as fp16 ~= exp(s)
# (multiplicative ripple up to +-3.6%; the softmax num/denom ratio cancels
# most of it -- measured 1.09e-2 end-to-end max rel vs the 2e-2 gate in v2).
A_EXP = float(2 ** 10 / np.log(2.0))
B_EXP = float(15.0 * 2 ** 10 - 292498.0 / 8192.0)  # centered ripple


def build_nc():
    nc = bacc.Bacc("TRN2", target_bir_lowering=False)

    t_query = nc.dram_tensor("query", [TQ, D], F32, kind="ExternalInput")
    t_key = nc.dram_tensor("key", [TLOC, D], F32, kind="ExternalInput")
    t_value = nc.dram_tensor("value", [TLOC, D], F32, kind="ExternalInput")
    t_wq = nc.dram_tensor("wq", [D, HD], F32, kind="ExternalInput")
    t_wk = nc.dram_tensor("wk", [D, HD], F32, kind="ExternalInput")
    t_wv = nc.dram_tensor("wv", [D, HD], F32, kind="ExternalInput")
    t_wo = nc.dram_tensor("wo", [HD, DOUT], F32, kind="ExternalInput")
    t_bq = nc.dram_tensor("bq", [HD], F32, kind="ExternalInput")
    t_bk = nc.dram_tensor("bk", [HD], F32, kind="ExternalInput")
    t_bv = nc.dram_tensor("bv", [HD], F32, kind="ExternalInput")
    t_bo = nc.dram_tensor("bo", [DOUT], F32, kind="ExternalInput")
    t_out = nc.dram_tensor("out", [TQ // NC_CORES, DOUT], F32, kind="ExternalOutput")

    with TileContext(nc) as tc:
        with tc.tile_pool(name="const", bufs=1) as constp, \
             tc.tile_pool(name="persist", bufs=1) as persist, \
             tc.tile_pool(name="ep", bufs=1) as ep, \
             tc.tile_pool(name="dram", bufs=1, space="DRAM") as dramp:

            ident = constp.tile([128, 128], F32)
            make_identity(nc, ident[:])

            # warm the ACT exp table before the main loop needs it
            dummy = constp.tile([128, 32], F32)
            nc.gpsimd.memset(dummy[:], 0.0)
            dexp = constp.tile([128, 32], F32R)
            nc.scalar.activation(dexp[:], dummy[:], EXP)

            # 0/1 block mask for the denominator broadcast matmul:
            # bcast4[j, 32j:32j+32] = 1 -> out[m, q] = rec[m//32, q].
            # Built as its transpose (memsets at 32-aligned partition bases,
            # which the ISA requires) and flipped once on the PE.
            bcT = constp.tile([128, 4], F32)
            nc.gpsimd.memset(bcT[:], 0.0)
            for j in range(4):
                nc.gpsimd.memset(bcT[32 * j:32 * (j + 1), j:j + 1], 1.0)
            bcast4 = constp.tile([4, 128], F32R)
            with tc.tile_pool(name="pBC", bufs=1, space="PSUM") as pBC:
                bc_ps = pBC.tile([4, 128], F32, tag="bcps", name="bcps")
                nc.tensor.transpose(bc_ps[:], bcT[:], ident[:])
                nc.vector.tensor_copy(bcast4[:], bc_ps[:])

            # persistent projected tensors (quad layout: tile m = heads
            # 4m..4m+3, head h at rows 32*(h%4)..)
            qhT = [persist.tile([128, TQ], F16, tag=f"qhT{m}", name=f"qhT{m}") for m in range(2)]
            khT = [persist.tile([128, TLOC], F16, tag=f"khT{m}", name=f"khT{m}") for m in range(2)]
            vh = [persist.tile([128, 256], F16, tag=f"vh{t}", name=f"vh{t}") for t in range(NKT)]
            # one shared all-ones column feeds every denominator matmul
            ones_col = persist.tile([128, 1], F16, tag="onescol", name="onescol")
            nc.gpsimd.memset(ones_col[:], 1.0)
            # epilogue accumulators (filled by per-segment RS-output DMAs)
            osum = [ep.tile([128, 256], F32, tag=f"osum{m}", name=f"osum{m}") for m in range(2)]
            # denominators packed [4 heads, t*256 + jq*64 + c]
            ldnP = ep.tile([4, 512], F32, tag="ldnP", name="ldnP")
            nc.gpsimd.memset(ldnP[:], 1.0)

            with tc.tile_pool(name="tin", bufs=6) as tin, \
                 tc.tile_pool(name="t16", bufs=6) as t16p, \
                 tc.tile_pool(name="tT", bufs=1) as tTp, \
                 tc.tile_pool(name="pS", bufs=3, space="PSUM") as pS, \
                 tc.tile_pool(name="pO", bufs=1, space="PSUM") as pO, \
                 tc.tile_pool(name="pD", bufs=1, space="PSUM") as pD, \
                 tc.tile_pool(name="pP", bufs=3) as pP, \
                 tc.tile_pool(name="stage", bufs=4) as stage:

                pre_raw = {}

                def load_raw(tdram, i, tag):
                    raw = tin.tile([128, 256], F32, tag=f"in_{tag}", name=f"in_{tag}")
                    nc.sync.dma_start(out=raw[:], in_=tdram[i * 128:(i + 1) * 128, :])
                    return raw

                for _i in range(4):
                    pre_raw[("kT", _i)] = load_raw(t_key, _i, "kT")

                # ---- weights + biases to SBUF (fp16; wq,bq pre-scaled by
                # 1/sqrt(KD)). One DMA per weight: [256,256] -> [128, 512]
                # with D-chunk a in cols 256a.. ----
                wcomb = {}
                with tc.tile_pool(name="wstage", bufs=2) as wstage:
                    for (tdram, key, scale_mul) in ((t_wk, "wk", None), (t_wq, "wq", SCALE),
                                                    (t_wv, "wv", None), (t_wo, "wo", None)):
                        raw = wstage.tile([128, 512], F32, tag="wraw", name="wraw")
                        nc.sync.dma_start(
                            out=raw[:].rearrange("p (a d) -> p a d", a=2),
                            in_=tdram[:].rearrange("(a p) d -> p a d", a=2))
                        wt = persist.tile([128, 512], F16, tag=f"w_{key}", name=f"w_{key}")
                        if scale_mul is not None:
                            nc.vector.tensor_scalar_mul(wt[:], raw[:], scale_mul)
                        else:
                            nc.vector.tensor_copy(wt[:], raw[:])
                        wcomb[key] = wt
                    wk_r = [wcomb["wk"][:, 256 * dc:256 * (dc + 1)] for dc in range(2)]
                    wq_r = [wcomb["wq"][:, 256 * dc:256 * (dc + 1)] for dc in range(2)]
                    wv_r = [wcomb["wv"][:, 256 * dc:256 * (dc + 1)] for dc in range(2)]
                    wo_r = [wcomb["wo"][:, 256 * dc:256 * (dc + 1)] for dc in range(2)]
                    for _i in range(4):
                        pre_raw[("qT", _i)] = load_raw(t_query, _i, "qT")
                    bq_c, bk_c, bo_c = [None, None], [None, None], [None, None]
                    for (tdram, dst, scale_mul, key) in ((t_bk, bk_c, None, "bk"),
                                                         (t_bq, bq_c, SCALE, "bq"),
                                                         (t_bo, bo_c, None, "bo")):
                        braw = wstage.tile([128, 2], F32, tag="braw", name="braw")
                        nc.sync.dma_start(out=braw[:],
                                          in_=tdram[:].rearrange("(a p) -> p a", a=2))
                        bt = persist.tile([128, 2], F32, tag=f"b_{key}", name=f"b_{key}")
                        if scale_mul is not None:
                            nc.vector.tensor_scalar_mul(bt[:], braw[:], scale_mul)
                        else:
                            nc.vector.tensor_copy(bt[:], braw[:])
                        for m in range(2):
                            dst[m] = bt[:, m:m + 1]
                    # bv replicated across partitions for the vh epilogue
                    bv_row = persist.tile([1, 256], F32)
                    nc.sync.dma_start(out=bv_row[:], in_=t_bv[:].rearrange("(a d) -> a d", a=1))
                    bv_rep = persist.tile([128, 256], F32)
                    nc.gpsimd.partition_broadcast(bv_rep[:], bv_row[0:1, :])

                qT = [tTp.tile([128, TQ], F16, tag=f"qT{m}", name=f"qT{m}") for m in range(2)]
                kT = [tTp.tile([128, TLOC], F16, tag=f"kT{m}", name=f"kT{m}") for m in range(2)]
                vT = [tTp.tile([128, TLOC], F16, tag=f"vT{m}", name=f"vT{m}") for m in range(2)]

                def load4_transpose(tdram, dst, j, tag):
                    """Load 4 raw [128,256] tiles (rows 512j..), transpose to
                    dst[dc][:, 512j:512j+512] via one pS tile of 8 transposes
                    + 2 strided copies (split across DVE/ACT). (An XBAR
                    transposing-DMA variant measured 1.7x SLOWER end-to-end:
                    the many small Q_I transfers arrive late and starve S.)"""
                    raws = []
                    for ti in range(4):
                        i = 4 * j + ti
                        raw = pre_raw.pop((tag, i), None)
                        if raw is None:
                            raw = load_raw(tdram, i, tag)
                        raws.append(raw)
                    tp = pS.tile([128, 1024], F32, tag="S", name="tpose")
                    for ti in range(4):
                        for m in range(2):
                            nc.tensor.transpose(tp[:, 128 * (2 * ti + m):128 * (2 * ti + m + 1)],
                                                raws[ti][:, m * 128:(m + 1) * 128], ident[:])
                    # column group for dc=m: indices {2ti+m} -> stride 256
                    tp4 = tp[:].rearrange("p (t m c) -> p t m c", t=4, m=2)
                    for m in range(2):
                        dv = dst[m][:, 512 * j:512 * (j + 1)].rearrange(
                            "p (t o c) -> p t o c", t=4, o=1)
                        if m == 0:
                            nc.vector.tensor_copy(dv, tp4[:, :, 0:1, :])
                        else:
                            nc.scalar.activation(dv, tp4[:, :, 1:2, :], COPY)

                def kchunk(j):
                    load4_transpose(t_key, kT, j, "kT")
                    pp = pS.tile([128, 1024], F32, tag="S", name="projk")
                    for m in range(2):
                        for dc in range(2):
                            nc.tensor.matmul(pp[:, m * 512:(m + 1) * 512],
                                             wk_r[dc][:, m * 128:(m + 1) * 128],
                                             kT[dc][:, j * 512:(j + 1) * 512],
                                             start=(dc == 0), stop=(dc == 1))
                    for m in range(2):
                        nc.vector.tensor_scalar_add(khT[m][:, j * 512:(j + 1) * 512],
                                                    pp[:, m * 512:(m + 1) * 512], bk_c[m])

                def qchunk(j):
                    load4_transpose(t_query, qT, j, "qT")
                    pp = pS.tile([128, 1024], F32, tag="S", name="projq")
                    for m in range(2):
                        for dc in range(2):
                            nc.tensor.matmul(pp[:, m * 512:(m + 1) * 512],
                                             wq_r[dc][:, m * 128:(m + 1) * 128],
                                             qT[dc][:, j * 512:(j + 1) * 512],
                                             start=(dc == 0), stop=(dc == 1))
                    for m in range(2):
                        nc.vector.tensor_scalar_add(qhT[m][:, j * 512:(j + 1) * 512],
                                                    pp[:, m * 512:(m + 1) * 512], bq_c[m])

                def vchunk(j):
                    load4_transpose(t_value, vT, j, "vT")
                    pp = pS.tile([128, 1024], F32, tag="S", name="projv")
                    for ti in range(4):
                        t = 4 * j + ti
                        for dc in range(2):
                            nc.tensor.matmul(pp[:, ti * 256:(ti + 1) * 256],
                                             vT[dc][:, t * 128:(t + 1) * 128],
                                             wv_r[dc][:], start=(dc == 0), stop=(dc == 1))
                    for ti in range(4):
                        t = 4 * j + ti
                        nc.vector.tensor_add(vh[t][:], pp[:, ti * 256:(ti + 1) * 256],
                                             bv_rep[:])

                # ---- main loop: 8 segments (jq, quad) x 16 kk, flat steps ----
                NSTEP = NJQ * 2 * NKT  # 128
                z_in = [dramp.tile([NC_CORES, 132, QG], F32, tag=f"zi{si}",
                                   name=f"zi{si}") for si in range(NJQ * 2)]
                z_out = [dramp.tile([132, QG], F32, tag=f"zo{si}",
                                    name=f"zo{si}") for si in range(NJQ * 2)]

                def step_seg(i):
                    return i // NKT  # segment index

                def seg_jq_t(si):
                    return si // 2, si % 2

                S_tiles = {}   # step -> (X, Y)
                P_tiles = {}   # step -> P
                psO_cur = [None]
                psD_cur = [None]

                def emit_S(i):
                    jq, t = seg_jq_t(step_seg(i))
                    kk = i % NKT
                    X = pS.tile([128, 1024], F32, tag="S", name="Sx")
                    Y = pS.tile([128, 1024], F32, tag="S", name="Sy")
                    for j in range(2):
                        nc.tensor.matmul(X[:, j * 512:(j + 1) * 512],
                                         khT[t][32 * j:32 * j + 32, kk * 128:(kk + 1) * 128],
                                         qhT[t][32 * j:32 * j + 32, jq * 512:(jq + 1) * 512],
                                         start=True, stop=True, tile_position=(32 * j, 0))
                    for j in range(2, 4):
                        nc.tensor.matmul(Y[:, (j - 2) * 512:(j - 1) * 512],
                                         khT[t][32 * j:32 * j + 32, kk * 128:(kk + 1) * 128],
                                         qhT[t][32 * j:32 * j + 32, jq * 512:(jq + 1) * 512],
                                         start=True, stop=True, tile_position=(32 * j, 0))
                    S_tiles[i] = (X, Y)

                def emit_exp(i):
                    X, Y = S_tiles.pop(i)
                    P = pP.tile([128, 2048], F16, tag="P", name="P")
                    if POOL_EXP:
                        nc.vector.tensor_scalar(
                            out=P[:, 0:XD].bitcast(I16), in0=X[:, 0:XD],
                            scalar1=A_EXP, scalar2=B_EXP,
                            op0=mybir.AluOpType.mult, op1=mybir.AluOpType.add)
                        nc.gpsimd.tensor_scalar(
                            out=P[:, XD:1024].bitcast(I16), in0=X[:, XD:1024],
                            scalar1=A_EXP, scalar2=B_EXP,
                            op0=mybir.AluOpType.mult, op1=mybir.AluOpType.add)
                        nc.gpsimd.tensor_scalar(
                            out=P[:, 1024:1024 + YP].bitcast(I16), in0=Y[:, 0:YP],
                            scalar1=A_EXP, scalar2=B_EXP,
                            op0=mybir.AluOpType.mult, op1=mybir.AluOpType.add)
                        nc.scalar.activation(P[:, 1024 + YP:2048], Y[:, YP:1024], EXP)
                    else:
                        nc.vector.tensor_scalar(
                            out=P[:, 0:1024].bitcast(I16), in0=X[:],
                            scalar1=A_EXP, scalar2=B_EXP,
                            op0=mybir.AluOpType.mult, op1=mybir.AluOpType.add)
                        nc.scalar.activation(P[:, 1024:2048], Y[:], EXP)
                    P_tiles[i] = P

                def emit_AV(i):
                    si = step_seg(i)
                    jq, t = seg_jq_t(si)
                    kk = i % NKT
                    first, last = kk == 0, kk == NKT - 1
                    if first:
                        psO_cur[0] = pO.tile([128, 512], F32, tag="psO", name="psO")
                        psD_cur[0] = pD.tile([128, 512], F32, tag="psD", name="psD")
                    P = P_tiles.pop(i)
                    psO, psD = psO_cur[0], psD_cur[0]
                    for j in range(4):
                        h = 4 * t + j
                        nc.tensor.matmul(psO[32 * j:32 * j + 32, :],
                                         vh[kk][:, 32 * h:32 * h + 32],
                                         P[:, j * 512:(j + 1) * 512],
                                         start=first, stop=last,
                                         tile_position=(0, 32 * j), skip_group_check=True)
                    for j in range(4):
                        nc.tensor.matmul(psD[32 * j:32 * j + 1, :],
                                         ones_col[:],
                                         P[:, j * 512:(j + 1) * 512],
                                         start=first, stop=last,
                                         tile_position=(0, 32 * j), skip_group_check=True)
                    return (psO, psD) if last else None

                def emit_drain(si, psO, psD):
                    stO = stage.tile([128, 512], F32, tag="stO", name="stO")
                    stD = stage.tile([128, 512], F32, tag="stD", name="stD")
                    # both drain copies on ACT: DVE is the steady-state
                    # bottleneck (fast-exp 1.23us/step), ACT has ~90ns slack
                    nc.scalar.activation(stO[:], psO[:], COPY)
                    nc.scalar.activation(stD[:], psD[:], COPY)
                    zi = z_in[si]
                    nc.sync.dma_start(
                        out=zi[:, 0:128, :].rearrange("r p c -> p r c"),
                        in_=stO[:].rearrange("p (r c) -> p r c", r=NC_CORES))
                    nc.sync.dma_start(
                        out=zi[:, 128:132, :].rearrange("r p c -> p r c"),
                        in_=stD[0:128:32, :].rearrange("p (r c) -> p r c", r=NC_CORES))
                    nc.gpsimd.collective_compute(
                        "ReduceScatter", mybir.AluOpType.add,
                        replica_groups=[list(range(NC_CORES))],
                        ins=[zi.opt()], outs=[z_out[si].opt()])

                # prologue chunks interleaved into the first steps: chunk
                # emitted at step i is consumed from step ~i+2 onward.
                prologue_at = {
                    0: lambda: kchunk(1), 1: lambda: vchunk(1),
                    2: lambda: kchunk(2), 3: lambda: vchunk(2),
                    5: lambda: kchunk(3), 7: lambda: vchunk(3),
                    10: lambda: qchunk(1), 16: lambda: qchunk(2),
                    24: lambda: qchunk(3),
                }

                kchunk(0)
                qchunk(0)
                emit_S(0)
                vchunk(0)
                pending = None
                for i in range(NSTEP):
                    if i in prologue_at:
                        prologue_at[i]()
                    emit_exp(i)
                    if i + 1 < NSTEP:
                        emit_S(i + 1)
                    if pending is not None:
                        emit_drain(*pending)
                        pending = None
                    fin = emit_AV(i)
                    if fin is not None:
                        pending = (step_seg(i), fin[0], fin[1])
                emit_drain(*pending)

            # ---- per-pair epilogue: as soon as segments (2jq, 2jq+1) have
            # been reduce-scattered, normalize + out-project + store the
            # 64 q rows this core owns in block jq. All of it is parked late
            # in the scheduler's model time so no queue orders it before
            # remaining main-loop work (head-of-line blocking). ----
            with tc.tile_pool(name="pRL", bufs=1, space="PSUM") as pRL, \
                 tc.tile_pool(name="pPO", bufs=1, space="PSUM") as pPO, \
                 tc.tile_pool(name="pPT", bufs=1, space="PSUM") as pPT, \
                 tc.tile_pool(name="epr", bufs=2) as epr:
                attnT = [ep.tile([128, 256], F16, tag=f"attnT{m}", name=f"attnT{m}")
                         for m in range(2)]
                for jq in range(NJQ):
                    with tc.tile_wait_until(0.24 + 0.001 * jq):
                        cols = slice(jq * QG, (jq + 1) * QG)
                        for t in range(2):
                            si = 2 * jq + t
                            zo = z_out[si]
                            nc.sync.dma_start(
                                out=osum[t][:, cols], in_=zo[0:128, :])
                            nc.sync.dma_start(
                                out=ldnP[0:4, 256 * t + jq * QG:256 * t + (jq + 1) * QG],
                                in_=zo[128:132, :])
                        # reciprocal of both quads' dens in one strided op,
                        # then one K=4 mask-matmul per quad broadcasts each
                        # head's recip row to its 32 (head,vd) output rows
                        rec = epr.tile([4, 128], F32R, tag="rec", name="rec")
                        with nc.allow_low_precision(reason="f32r recip feeds matmul"):
                            nc.vector.reciprocal(
                                rec[:].rearrange("p (t c) -> p t c", t=2),
                                ldnP[:].rearrange("p (t j c) -> p t j c", t=2, j=NJQ)[:, :, jq, :])
                        rlP = [pRL.tile([128, 64], F32, tag=f"rlP{t}", name=f"rlP{t}")
                               for t in range(2)]
                        for t in range(2):
                            nc.tensor.matmul(rlP[t][:], bcast4[:],
                                             rec[:, 64 * t:64 * (t + 1)],
                                             start=True, stop=True)
                        for t in range(2):
                            nc.vector.tensor_mul(attnT[t][:, cols],
                                                 osum[t][:, cols], rlP[t][:])
                        # out-proj for this q block: one PSUM tile per dout
                        # chunk; m=0,1 accumulate into it.
                        pout = [pPO.tile([128, 64], F32, tag=f"pout{dc}", name=f"pout{dc}")
                                for dc in range(2)]
                        for dc in range(2):
                            for m in range(2):
                                nc.tensor.matmul(pout[dc][:],
                                                 wo_r[m][:, dc * 128:(dc + 1) * 128],
                                                 attnT[m][:, cols], start=(m == 0),
                                                 stop=(m == 1), skip_group_check=True)
                        oT = epr.tile([128, 128], F32, tag="oT", name="oT")
                        for dc in range(2):
                            nc.vector.tensor_scalar_add(oT[:, 64 * dc:64 * (dc + 1)],
                                                        pout[dc][:], bo_c[dc])
                        pt = [pPT.tile([64, 128], F32, tag=f"pt{dc}", name=f"pt{dc}")
                              for dc in range(2)]
                        for dc in range(2):
                            nc.tensor.transpose(pt[dc][:],
                                                oT[:, 64 * dc:64 * (dc + 1)], ident[:])
                        out_sb = epr.tile([64, 256], F32, tag="outsb", name="outsb")
                        for dc in range(2):
                            nc.vector.tensor_copy(out_sb[:, 128 * dc:128 * (dc + 1)],
                                                  pt[dc][:])
                        nc.sync.dma_start(out=t_out[jq * QG:(jq + 1) * QG, :],
                                          in_=out_sb[:])

    nc.compile()
    return nc


_NC_CACHE = {}


def _get_nc():
    if "nc" not in _NC_CACHE:
        _NC_CACHE["nc"] = build_nc()
    return _NC_CACHE["nc"]


def run_cores(inputs, trace=False):
    nc = _get_nc()
    full = {k: np.ascontiguousarray(np.asarray(v, dtype=np.float32)) for k, v in inputs.items()}
    in_maps = []
    for c in range(NC_CORES):
        m = dict(full)
        m["key"] = np.ascontiguousarray(full["key"][c * TLOC:(c + 1) * TLOC])
        m["value"] = np.ascontiguousarray(full["value"][c * TLOC:(c + 1) * TLOC])
        in_maps.append(m)
    res = run_bass_kernel_spmd(nc, in_maps, core_ids=list(range(NC_CORES)), trace=trace)
    out = np.empty((TQ, DOUT), dtype=np.float32)
    for r in range(NC_CORES):
        blk = res.results[r]["out"]
        for jq in range(NJQ):
            q0 = QG * (NC_CORES * jq + r)
            out[q0:q0 + QG, :] = blk[QG * jq:QG * (jq + 1), :]
    return out, res


def kernel(**inputs) -> np.ndarray:
    out, _ = run_cores(inputs, trace=False)
    return out
